# revision 14
# baseline (speedup 1.0000x reference)
"""BailingMoE Trainium2 kernel (8-core SPMD, expert-parallel).

Sharding: 2 experts per core (E=16 over 8 cores), shared-expert MLP
tensor-parallel on the intermediate dim (IS=2816 -> 352/core), router
replicated.  Each core:
  - computes the router (logits -> exp -> top4 threshold -> renormalized
    combine weights) on device,
  - gathers its experts' routed tokens (capacity C) via indirect DMA,
  - runs the expert MLPs on just those tokens (i-major grouped GEMM),
  - scatters weighted expert outputs back with DMA-accumulate,
  - computes its shard of the shared MLP densely (token-major out).
Host sums the per-core partials.

Weights are pre-laid-out on the host (transposes/reorders only: data is
bit-identical fp32) so every device DMA is a large mostly-contiguous read.
"""

import os
import sys
import numpy as np
import ml_dtypes
from contextlib import ExitStack

sys.path.insert(0, "/opt/trn_rl_repo")

# ---- problem constants (hardcoded per contest rules) ----
T = 1024
H = 2048
E = 16
TOPK = 4
I = 1408
IS = 2816          # shared intermediate
NCORES = 8
EPC = 2            # experts per core
SHARD = IS // NCORES          # 352 shared-intermediate per core
SHARD_PAD = 384               # padded to 3*128
C = 384                       # per-expert token capacity (max count + margin)
P = 128
KH = H // P        # 16  contraction tiles over H
NT = T // P        # 8   token tiles
MG = I // P        # 11  gate i-tiles per expert (up tiles are MG..2*MG-1)
NB = C // P        # 3   gather batches per expert
MSP = SHARD_PAD // P  # 3  shared gate tiles (up at +3)
KD = SHARD_PAD // P   # 3  shared-down contraction tiles
HC = 4             # output H chunks of 512
HCW = H // HC      # 512

_CACHED = {}


def _host_routing(x, Wg):
    """Replicates the reference router exactly (fp64-free, fp32 math)."""
    logits = (x.astype(np.float32) @ Wg.astype(np.float32)).astype(np.float32)
    m = logits.max(axis=-1, keepdims=True)
    ev = np.exp(logits - m)
    probs = ev / ev.sum(axis=-1, keepdims=True)
    order = np.argsort(-probs, axis=-1, kind="stable")[:, :TOPK]
    topw = np.take_along_axis(probs, order, axis=-1)
    topw = topw / topw.sum(axis=-1, keepdims=True)
    combine = np.zeros((T, E), dtype=np.float32)
    np.put_along_axis(combine, order, topw.astype(np.float32), axis=-1)
    return combine


def _layout_inputs(inputs):
    """Build the 8 per-core input maps (host-side shard + re-layout)."""
    x = np.ascontiguousarray(inputs["x"], dtype=np.float32)
    Wg = np.ascontiguousarray(inputs["Wg"], dtype=np.float32)
    W1 = np.ascontiguousarray(inputs["W1"], dtype=np.float32)
    W2 = np.ascontiguousarray(inputs["W2"], dtype=np.float32)
    Wsg = np.ascontiguousarray(inputs["Wsg"], dtype=np.float32)
    Wsd = np.ascontiguousarray(inputs["Wsd"], dtype=np.float32)

    BF = ml_dtypes.bfloat16
    xT = np.ascontiguousarray(x.T)                       # [H, T] fp32 (router)
    xTb = np.ascontiguousarray(x.T.astype(BF))           # [H, T] bf16 (mm)
    xpad = np.zeros((T + 1, H), dtype=BF)
    xpad[:T] = x.astype(BF)

    combine = _host_routing(x, Wg)

    WsgT = np.ascontiguousarray(Wsg.T)                   # [H, 2*IS]
    WsdT = np.ascontiguousarray(Wsd.T)                   # [IS, H]

    iota16 = np.arange(T, dtype=np.float32).reshape(T // 16, 16).T
    iota16 = np.ascontiguousarray(iota16)                # [16, 64]: f*16+p

    in_maps = []
    for c in range(NCORES):
        m = {"xT": xT, "xTb": xTb, "xpad": xpad, "Wg": Wg,
             "ident": np.eye(P, dtype=BF), "iota16": iota16}
        esel = np.zeros((P, EPC * E), dtype=np.float32)
        for le in range(EPC):
            esel[:, le * E + c * EPC + le] = 1.0
        m["esel"] = esel

        # --- expert weights, host re-laid for single-DMA pair blocks ---
        for le in range(EPC):
            e = c * EPC + le
            W1T_e = np.ascontiguousarray(W1[e].T)        # [H, 2I]
            # [pair, k, p, 256] : gate col-block mg | up col-block mg+MG
            r = W1T_e.reshape(KH, P, 2 * MG, P).transpose(2, 0, 1, 3)
            w1p = np.concatenate([r[:MG], r[MG:]], axis=-1)
            m[f"w1p{le}"] = np.ascontiguousarray(
                w1p.reshape(MG * KH * P, 2 * P).astype(BF))
            W2T_e = np.ascontiguousarray(W2[e].T)        # [I, H]
            r2 = W2T_e.reshape(MG, P, HC, HCW).transpose(2, 0, 1, 3)
            m[f"w2p{le}"] = np.ascontiguousarray(
                r2.reshape(HC * MG * P, HCW).astype(BF))

        # --- shared MLP shard (gate/up cols padded 352->384) ---
        gs = WsgT[:, c * SHARD:(c + 1) * SHARD]
        us = WsgT[:, IS + c * SHARD: IS + (c + 1) * SHARD]
        wsg_pad = np.zeros((H, 2 * SHARD_PAD), dtype=np.float32)
        wsg_pad[:, :SHARD] = gs
        wsg_pad[:, SHARD_PAD:SHARD_PAD + SHARD] = us
        rs = wsg_pad.reshape(KH, P, 2 * MSP, P).transpose(2, 0, 1, 3)
        wsgp = np.concatenate([rs[:MSP], rs[MSP:]], axis=-1)
        m["wsgp"] = np.ascontiguousarray(
            wsgp.reshape(MSP * KH * P, 2 * P).astype(BF))

        wsd_pad = np.zeros((SHARD_PAD, H), dtype=np.float32)
        wsd_pad[:SHARD] = WsdT[c * SHARD:(c + 1) * SHARD]
        rd = wsd_pad.reshape(KD, P, HC, HCW).transpose(2, 0, 1, 3)
        m["wsdp"] = np.ascontiguousarray(
            rd.reshape(HC * KD * P, HCW).astype(BF))

        # --- routed token slots (host routing; device validates combine) ---
        sidx = np.full((EPC, C), T, dtype=np.int32)      # pad -> zero row
        sw = np.zeros((EPC, C), dtype=np.float32)
        for le in range(EPC):
            e = c * EPC + le
            sel = np.nonzero(combine[:, e] > 0)[0]
            assert len(sel) <= C, f"capacity overflow: {len(sel)} > {C}"
            sidx[le, :len(sel)] = sel
            sw[le, :len(sel)] = combine[sel, e]
        m["sidx"] = sidx.reshape(EPC * NB, P)
        m["sw"] = sw.reshape(EPC * NB, P)
        in_maps.append(m)
    return in_maps, combine


def build_program(ondev_routing=True):
    from concourse import bacc, mybir, tile, bass

    dt = mybir.dt
    f32 = dt.float32
    bf16 = dt.bfloat16
    AF = mybir.ActivationFunctionType
    OP = mybir.AluOpType
    AX = mybir.AxisListType

    nc = bacc.Bacc("TRN2", target_bir_lowering=False, debug=False)

    def din(name, shape, dtype=f32):
        return nc.dram_tensor(name, shape, dtype, kind="ExternalInput").ap()

    xT = din("xT", [H, T])
    xTb = din("xTb", [H, T], bf16)
    xpad = din("xpad", [T + 1, H], bf16)
    Wg = din("Wg", [H, E])
    ident = din("ident", [P, P], bf16)
    w1p = [din(f"w1p{le}", [MG * KH * P, 2 * P], bf16) for le in range(EPC)]
    w2p = [din(f"w2p{le}", [HC * MG * P, HCW], bf16) for le in range(EPC)]
    wsgp = din("wsgp", [MSP * KH * P, 2 * P], bf16)
    wsdp = din("wsdp", [HC * KD * P, HCW], bf16)
    sidx = din("sidx", [EPC * NB, P], dt.int32)
    sw = din("sw", [EPC * NB, P])
    iota16 = din("iota16", [16, T // 16])
    esel = din("esel", [P, EPC * E])

    out_s = nc.dram_tensor("out_s", [T, H], f32, kind="ExternalOutput").ap()
    out_m = nc.dram_tensor("out_m", [T + 1, H], f32,
                           kind="ExternalOutput").ap()
    comb_out = nc.dram_tensor("comb", [T, E], f32, kind="ExternalOutput").ap()

    with tile.TileContext(nc) as tc, ExitStack() as ctx:
        cpool = ctx.enter_context(tc.tile_pool(name="const", bufs=1))
        psum = ctx.enter_context(
            tc.tile_pool(name="ps", bufs=8, space="PSUM"))
        outp = ctx.enter_context(tc.tile_pool(name="outp", bufs=4))

        ident_sb = cpool.tile([P, P], bf16)
        nc.sync.dma_start(ident_sb[:], ident[:, :])

        idx_sb = cpool.tile([P, EPC * NB], dt.int32)
        w_sb = cpool.tile([P, EPC * NB], f32)
        if not ondev_routing:
            for r in range(EPC * NB):
                nc.sync.dma_start(idx_sb[:, r:r + 1],
                                  sidx[r, :].rearrange("(p o) -> p o", o=1))
                nc.sync.dma_start(w_sb[:, r:r + 1],
                                  sw[r, :].rearrange("(p o) -> p o", o=1))

        # =========== stage 1: xT resident; router; shared mm1 ===========
        a_s = cpool.tile([P, MSP * T], bf16)    # shared act [3 tiles x T]
        a_e = [cpool.tile([P, MG * C], bf16, name=f"a_e{le}")
               for le in range(EPC)]
        comb_sb = cpool.tile([P, NT * E], f32)

        with tc.tile_pool(name="xt", bufs=1) as xt_pool, \
             tc.tile_pool(name="rt", bufs=2) as rt, \
             tc.tile_pool(name="wst1", bufs=2) as wst:
            xt_sb = xt_pool.tile([P, KH * T], f32)
            nc.sync.dma_start(
                xt_sb[:].rearrange("p (k t) -> p k t", k=KH),
                xT[:, :].rearrange("(k p) t -> p k t", p=P))
            xtb_sb = xt_pool.tile([P, KH * T], bf16)
            nc.sync.dma_start(
                xtb_sb[:].rearrange("p (k t) -> p k t", k=KH),
                xTb[:, :].rearrange("(k p) t -> p k t", p=P))

            wg_sb = rt.tile([P, KH * E], f32)
            nc.sync.dma_start(
                wg_sb[:].rearrange("p (k e) -> p k e", k=KH),
                Wg[:, :].rearrange("(k p) e -> p k e", p=P))

            # ---- router: logits per token tile (fp32 for exactness) ----
            logits = rt.tile([P, NT * E], f32)
            for t in range(NT):
                ps = psum.tile([P, E], f32, tag="ps")
                for k in range(KH):
                    nc.tensor.matmul(
                        ps[:],
                        lhsT=xt_sb[:, k * T + t * P: k * T + (t + 1) * P],
                        rhs=wg_sb[:, k * E:(k + 1) * E],
                        start=(k == 0), stop=(k == KH - 1))
                nc.scalar.copy(logits[:, t * E:(t + 1) * E], ps[:])

            ev = rt.tile([P, NT * E], f32)
            nc.scalar.activation(ev[:], logits[:], AF.Exp)
            buf = rt.tile([P, NT * E], f32)
            nc.vector.tensor_copy(buf[:], ev[:])
            b3 = buf[:].rearrange("p (t e) -> p t e", t=NT)
            mx = rt.tile([P, NT], f32)
            msk = rt.tile([P, NT * E], f32)
            m3 = msk[:].rearrange("p (t e) -> p t e", t=NT)
            for r in range(TOPK - 1):
                nc.vector.tensor_reduce(mx[:], b3, axis=AX.X, op=OP.max)
                mxb = mx[:].rearrange("p (t o) -> p t o", o=1) \
                           .to_broadcast([P, NT, E])
                nc.vector.tensor_tensor(m3, b3, mxb, op=OP.is_ge)
                nc.vector.scalar_tensor_tensor(
                    b3, m3, -1e30, b3, op0=OP.mult, op1=OP.add)
            nc.vector.tensor_reduce(mx[:], b3, axis=AX.X, op=OP.max)
            mxb = mx[:].rearrange("p (t o) -> p t o", o=1) \
                       .to_broadcast([P, NT, E])
            e3 = ev[:].rearrange("p (t e) -> p t e", t=NT)
            nc.vector.tensor_tensor(m3, e3, mxb, op=OP.is_ge)
            evm = rt.tile([P, NT * E], f32)
            evm3 = evm[:].rearrange("p (t e) -> p t e", t=NT)
            nc.vector.tensor_tensor(evm3, e3, m3, op=OP.mult)
            den = rt.tile([P, NT], f32)
            nc.vector.tensor_reduce(den[:], evm3, axis=AX.X, op=OP.add)
            inv = rt.tile([P, NT], f32)
            nc.vector.reciprocal(inv[:], den[:])
            invb = inv[:].rearrange("p (t o) -> p t o", o=1) \
                         .to_broadcast([P, NT, E])
            c3 = comb_sb[:].rearrange("p (t e) -> p t e", t=NT)
            nc.vector.tensor_tensor(c3, evm3, invb, op=OP.mult)
            for t in range(NT):
                nc.sync.dma_start(comb_out[t * P:(t + 1) * P, :],
                                  comb_sb[:, t * E:(t + 1) * E])

            # ---- shared mm1 (i-major): pairs (m, m+3) ----
            for m in range(MSP):
                wt = wst.tile([P, KH * 2 * P], bf16, tag="w")
                nc.sync.dma_start(
                    wt[:].rearrange("p (k c) -> p k c", k=KH),
                    wsgp[m * KH * P:(m + 1) * KH * P, :]
                    .rearrange("(k p) c -> p k c", p=P))
                for n in range(2):
                    pg = psum.tile([P, HCW], f32, tag="ps")
                    pu = psum.tile([P, HCW], f32, tag="ps")
                    for k in range(KH):
                        mv = xtb_sb[:, k * T + n * HCW:
                                    k * T + (n + 1) * HCW]
                        nc.tensor.matmul(
                            pg[:], lhsT=wt[:, k * 2 * P: k * 2 * P + P],
                            rhs=mv, start=(k == 0), stop=(k == KH - 1))
                        nc.tensor.matmul(
                            pu[:], lhsT=wt[:, k * 2 * P + P:
                                           (k + 1) * 2 * P],
                            rhs=mv, start=(k == 0), stop=(k == KH - 1))
                    sg = rt.tile([P, HCW], f32, tag="sg")
                    nc.scalar.activation(sg[:], pg[:], AF.Sigmoid)
                    nc.vector.tensor_tensor(sg[:], sg[:], pg[:], op=OP.mult)
                    nc.vector.tensor_tensor(
                        a_s[:, m * T + n * HCW: m * T + (n + 1) * HCW],
                        sg[:], pu[:], op=OP.mult)

        # ====== stage 1.5: on-device routed-slot compaction ======
        if ondev_routing:
            NF = (T + C) // 16           # 88: 64 token cols + 24 sentinel
            CF = C // 16                 # 24 compacted cols
            TF = T // 16                 # 64
            with tc.tile_pool(name="cmp", bufs=2) as cmp, \
                 tc.tile_pool(name="cmc", bufs=1) as cmc, \
                 tc.tile_pool(name="cdram", bufs=1, space="DRAM") as cdram:
                iota_sb = cmc.tile([16, TF], f32)
                nc.sync.dma_start(iota_sb[:], iota16[:, :])
                esel_sb = cmc.tile([P, EPC * E], f32)
                nc.sync.dma_start(esel_sb[:], esel[:, :])
                negones = cmc.tile([16, TF], f32)
                nc.gpsimd.memset(negones[:], -1.0)
                comb_loc = cdram.tile([T, EPC], f32)

                c3v = comb_sb[:].rearrange("p (t e) -> p t e", t=NT)
                for le in range(EPC):
                    # extract this core's expert column via one-hot (data!)
                    selb = esel_sb[:, le * E:(le + 1) * E] \
                        .rearrange("p (o e) -> p o e", o=1) \
                        .to_broadcast([P, NT, E])
                    tmp = cmp.tile([P, NT * E], f32, tag="tmp")
                    t3 = tmp[:].rearrange("p (t e) -> p t e", t=NT)
                    nc.vector.tensor_tensor(t3, c3v, selb, op=OP.mult)
                    colmap = cmp.tile([P, NT], f32, tag="colmap")
                    nc.vector.tensor_reduce(colmap[:], t3, axis=AX.X,
                                            op=OP.add)
                    nc.sync.dma_start(
                        comb_loc[:, le:le + 1]
                        .rearrange("(t p) o -> p (t o)", p=P),
                        colmap[:])

                for le in range(EPC):
                    combcol = cmp.tile([16, TF], f32, tag="cc")
                    nc.sync.dma_start(
                        combcol[:],
                        comb_loc[:, le:le + 1]
                        .rearrange("(f p) o -> p (f o)", p=16))
                    mgt = cmp.tile([16, TF], dt.int32, tag="mgt")
                    nc.vector.tensor_scalar(
                        mgt[:], combcol[:], 0.0, None, op0=OP.is_gt)
                    v = cmp.tile([16, NF], f32, tag="v")
                    nc.gpsimd.memset(v[:, TF:], float(T))
                    nc.vector.tensor_copy(v[:, :TF], negones[:])
                    nc.vector.copy_predicated(v[:, :TF], mgt[:], iota_sb[:])
                    vw = cmp.tile([16, NF], f32, tag="vw")
                    nc.gpsimd.memset(vw[:, TF:], 0.0)
                    nc.vector.tensor_copy(vw[:, :TF], negones[:])
                    nc.vector.copy_predicated(vw[:, :TF], mgt[:], combcol[:])

                    ids_c = cmp.tile([16, NF], f32, tag="ids_c")
                    w_c = cmp.tile([16, NF], f32, tag="w_c")
                    nf1 = cmp.tile([1, 1], dt.uint32, tag="nf1")
                    nf2 = cmp.tile([1, 1], dt.uint32, tag="nf2")
                    nc.gpsimd.sparse_gather(ids_c[:], v[:], num_found=nf1[:])
                    nc.gpsimd.sparse_gather(w_c[:], vw[:], num_found=nf2[:])

                    ids_d = cdram.tile([16, CF], f32, tag="ids_d")
                    w_d = cdram.tile([16, CF], f32, tag="w_d")
                    nc.sync.dma_start(ids_d[:, :], ids_c[:, :CF])
                    nc.sync.dma_start(w_d[:, :], w_c[:, :CF])
                    for b in range(NB):
                        idf = cmp.tile([P, 1], f32, tag="idf")
                        srci = ids_d[:, :] \
                            .rearrange("r (b a) -> b a r", b=NB)[b]
                        nc.sync.dma_start(idf[:], srci)
                        nc.vector.tensor_copy(
                            idx_sb[:, le * NB + b: le * NB + b + 1], idf[:])
                        srcw = w_d[:, :] \
                            .rearrange("r (b a) -> b a r", b=NB)[b]
                        nc.sync.dma_start(
                            w_sb[:, le * NB + b: le * NB + b + 1], srcw)

        # =========== stage 2: gather + transpose + expert mm1 ===========
        with tc.tile_pool(name="gat", bufs=2) as gat, \
             tc.tile_pool(name="xet", bufs=2) as xet_pool, \
             tc.tile_pool(name="wst2", bufs=2) as wst2:
            for le in range(EPC):
                xeT = xet_pool.tile([P, KH * C], bf16, tag="xeT")
                for b in range(NB):
                    xe = gat.tile([P, H], bf16, tag="xe")
                    nc.gpsimd.indirect_dma_start(
                        out=xe[:], out_offset=None,
                        in_=xpad[:, :],
                        in_offset=bass.IndirectOffsetOnAxis(
                            ap=idx_sb[:, le * NB + b: le * NB + b + 1],
                            axis=0))
                    for k in range(KH):
                        pt = psum.tile([P, P], bf16, tag="ps")
                        nc.tensor.transpose(
                            pt[:], xe[:, k * P:(k + 1) * P], ident_sb[:])
                        nc.vector.tensor_copy(
                            xeT[:, k * C + b * P: k * C + (b + 1) * P],
                            pt[:])

                for m in range(MG):
                    wt = wst2.tile([P, KH * 2 * P], bf16, tag="w")
                    nc.sync.dma_start(
                        wt[:].rearrange("p (k c) -> p k c", k=KH),
                        w1p[le][m * KH * P:(m + 1) * KH * P, :]
                        .rearrange("(k p) c -> p k c", p=P))
                    pg = psum.tile([P, C], f32, tag="ps")
                    pu = psum.tile([P, C], f32, tag="ps")
                    for k in range(KH):
                        mv = xeT[:, k * C:(k + 1) * C]
                        nc.tensor.matmul(
                            pg[:], lhsT=wt[:, k * 2 * P: k * 2 * P + P],
                            rhs=mv, start=(k == 0), stop=(k == KH - 1))
                        nc.tensor.matmul(
                            pu[:], lhsT=wt[:, k * 2 * P + P:
                                           (k + 1) * 2 * P],
                            rhs=mv, start=(k == 0), stop=(k == KH - 1))
                    sg = gat.tile([P, C], f32, tag="sg")
                    nc.scalar.activation(sg[:], pg[:], AF.Sigmoid)
                    nc.vector.tensor_tensor(sg[:], sg[:], pg[:], op=OP.mult)
                    nc.vector.tensor_tensor(
                        a_e[le][:, m * C:(m + 1) * C], sg[:], pu[:],
                        op=OP.mult)

        # =========== stage 3: shared mm2 (token-major dense) ===========
        with tc.tile_pool(name="wst3", bufs=2) as wst3:
          for hc in range(HC):
            wd = wst3.tile([P, KD * HCW], bf16, tag="wd")
            nc.sync.dma_start(
                wd[:].rearrange("p (k c) -> p k c", k=KD),
                wsdp[hc * KD * P:(hc + 1) * KD * P, :]
                .rearrange("(k p) c -> p k c", p=P))
            for tg in range(2):
                pss = [psum.tile([P, HCW], f32, tag="ps", name=f"pss{hc}_{tg}_{i}")
                       for i in range(4)]
                for k in range(KD):
                    for tt in range(4):
                        tau = tg * 4 + tt
                        nc.tensor.matmul(
                            pss[tt][:],
                            lhsT=a_s[:, k * T + tau * P:
                                     k * T + (tau + 1) * P],
                            rhs=wd[:, k * HCW:(k + 1) * HCW],
                            start=(k == 0), stop=(k == KD - 1))
                for tt in range(4):
                    tau = tg * 4 + tt
                    ob = outp.tile([P, HCW], f32, tag="ob")
                    nc.any.tensor_copy(ob[:], pss[tt][:])
                    nc.sync.dma_start(
                        out_s[tau * P:(tau + 1) * P,
                              hc * HCW:(hc + 1) * HCW], ob[:])

        # =========== stage 4: expert mm2 + weighted scatter-add ===========
        with tc.tile_pool(name="wst4", bufs=2) as wst4:
          for le in range(EPC):
            for hc in range(HC):
                wd = wst4.tile([P, MG * HCW], bf16, tag="wd2")
                nc.sync.dma_start(
                    wd[:].rearrange("p (k c) -> p k c", k=MG),
                    w2p[le][hc * MG * P:(hc + 1) * MG * P, :]
                    .rearrange("(k p) c -> p k c", p=P))
                pse = [psum.tile([P, HCW], f32, tag="ps", name=f"pse{le}_{hc}_{i}")
                       for i in range(NB)]
                for k in range(MG):
                    for b in range(NB):
                        nc.tensor.matmul(
                            pse[b][:],
                            lhsT=a_e[le][:, k * C + b * P:
                                         k * C + (b + 1) * P],
                            rhs=wd[:, k * HCW:(k + 1) * HCW],
                            start=(k == 0), stop=(k == MG - 1))
                for b in range(NB):
                    yb = outp.tile([P, HCW], f32, tag="yb")
                    nc.vector.tensor_scalar_mul(
                        yb[:], pse[b][:],
                        scalar1=w_sb[:, le * NB + b: le * NB + b + 1])
                    nc.gpsimd.indirect_dma_start(
                        out=out_m[:, :],
                        out_offset=bass.IndirectOffsetOnAxis(
                            ap=idx_sb[:, le * NB + b: le * NB + b + 1],
                            axis=0),
                        in_=yb[:], in_offset=None,
                        element_offset=hc * HCW,
                        compute_op=mybir.AluOpType.add)

    nc.compile()
    return nc


def get_program():
    ondev = os.environ.get("BAILING_HOST_ROUTING") != "1"
    key = ("nc", ondev)
    if key not in _CACHED:
        _CACHED[key] = build_program(ondev_routing=ondev)
    return _CACHED[key]


def _get_runner():
    """Build (once) a cached PJRT executable over 8 cores.

    Mirrors concourse.bass2jax.run_bass_via_pjrt but keeps the jitted
    callable so repeated kernel() calls skip the multi-minute NEFF
    compile.  Outputs are donated fresh zero buffers each call (the
    scatter-accumulate path relies on zero-initialized out_m).
    """
    if "runner" in _CACHED:
        return _CACHED["runner"]
    import jax
    from jax.sharding import Mesh, PartitionSpec, NamedSharding
    from jax.experimental.shard_map import shard_map
    from concourse import mybir
    from concourse.bass2jax import (
        install_neuronx_cc_hook, _bass_exec_p, partition_id_tensor)

    install_neuronx_cc_hook()
    nc = get_program()
    partition_name = (nc.partition_id_tensor.name
                      if nc.partition_id_tensor else None)
    in_names, out_names, out_avals, zero_outs = [], [], [], []
    for alloc in nc.m.functions[0].allocations:
        if not isinstance(alloc, mybir.MemoryLocationSet):
            continue
        name = alloc.memorylocations[0].name
        if alloc.kind == "ExternalInput":
            if name != partition_name:
                in_names.append(name)
        elif alloc.kind == "ExternalOutput":
            out_names.append(name)
            shape = tuple(alloc.tensor_shape)
            dtype = mybir.dt.np(alloc.dtype)
            out_avals.append(jax.core.ShapedArray(shape, dtype))
            zero_outs.append(np.zeros(shape, dtype))
    n_params = len(in_names)
    n_outs = len(out_avals)
    all_in = list(in_names) + list(out_names)
    if partition_name is not None:
        all_in.append(partition_name)

    def _body(*args):
        operands = list(args)
        if partition_name is not None:
            operands.append(partition_id_tensor())
        return tuple(_bass_exec_p.bind(
            *operands, out_avals=tuple(out_avals), in_names=tuple(all_in),
            out_names=tuple(out_names), lowering_input_output_aliases=(),
            sim_require_finite=True, sim_require_nnan=True, nc=nc))

    devices = jax.devices()[:NCORES]
    mesh = Mesh(np.asarray(devices), ("core",))
    fn = jax.jit(
        shard_map(_body, mesh=mesh,
                  in_specs=(PartitionSpec("core"),) * (n_params + n_outs),
                  out_specs=(PartitionSpec("core"),) * n_outs,
                  check_rep=False),
        donate_argnums=tuple(range(n_params, n_params + n_outs)),
        keep_unused=True)
    sharding = NamedSharding(mesh, PartitionSpec("core"))
    runner = (fn, in_names, out_names, zero_outs, sharding)
    _CACHED["runner"] = runner
    return runner


def kernel(**inputs):
    import jax

    in_maps, _ = _layout_inputs(inputs)
    fn, in_names, out_names, zero_outs, sharding = _get_runner()
    gargs = []
    for name in in_names:
        g = np.concatenate([np.asarray(m[name]) for m in in_maps], axis=0)
        gargs.append(jax.device_put(g, sharding))
    for z in zero_outs:
        gargs.append(jax.device_put(
            np.concatenate([z] * NCORES, axis=0), sharding))
    outs = fn(*gargs)
    om = {n: np.asarray(outs[i]) for i, n in enumerate(out_names)}
    s = om["out_s"].reshape(NCORES, T, H)
    m = om["out_m"].reshape(NCORES, T + 1, H)
    out = s.sum(axis=0) + m[:, :T].sum(axis=0)
    return out.astype(inputs["x"].dtype)


# ---------- numpy model of one core's partials (for testing) ----------
def core_partials_numpy(inputs, core):
    x = inputs["x"].astype(np.float32)
    combine = _host_routing(x, inputs["Wg"].astype(np.float32))
    W1, W2 = inputs["W1"], inputs["W2"]
    Wsg, Wsd = inputs["Wsg"], inputs["Wsd"]

    def silu(v):
        return v / (1.0 + np.exp(-v))

    gs = Wsg[core * SHARD:(core + 1) * SHARD]
    us = Wsg[IS + core * SHARD: IS + (core + 1) * SHARD]
    hs = silu(x @ gs.T) * (x @ us.T)
    out_s = hs @ Wsd[:, core * SHARD:(core + 1) * SHARD].T

    out_m = np.zeros((T + 1, H), dtype=np.float32)
    for le in range(EPC):
        e = core * EPC + le
        sel = np.nonzero(combine[:, e] > 0)[0]
        xe = x[sel]
        h = xe @ W1[e].T
        a = silu(h[:, :I]) * h[:, I:]
        y = (a @ W2[e].T) * combine[sel, e][:, None]
        out_m[sel] += y
    return out_s.astype(np.float32), out_m, combine
